# revision 1
# baseline (speedup 1.0000x reference)
"""Trainium2 Bass kernel for BatteryMoEFlattenIntraCycleMoELayer.

Computation (reference):
    gates = renorm(top2(softmax(logits) * mask))          # [B, E]
    x = cycle_curve_data.reshape(B, L, 900)
    out[b] = sum_e gates[b,e] * (x[b] @ W[e] + b[e])      # -> bf16 [B, L, 512]

Strategy:
  - Host: compute gates + top-2 routing (tiny), transpose x to feat-major
    [B, 901, 128] with a constant-1.0 row appended (folds the bias add into
    the matmul via weight augmentation W_aug = [W; b]).
  - Shard B across 8 cores (64 samples each). One SPMD program: routing is
    carried as *data* (per-sample W-slot element offsets, read into PE
    registers at runtime -> dynamic access patterns on the matmul moving
    operand), so the program does not depend on input values.
  - Device per sample: 2 experts x 8 K-chunks matmuls (N=512, float32r at
    full PE rate) accumulate x_aug @ W_aug[e] into 2 PSUM banks; ACT engine
    scales each by its gate (per-partition scalar AP from data); DVE adds
    and casts to bf16. A k-outer phase over the first 12 samples overlaps
    the 16.8 MB weight load with compute.
"""

import os
import sys

for _p in ("/opt/trn_rl_repo", "/root/.axon_site/_ro/trn_rl_repo"):
    if os.path.isdir(_p) and _p not in sys.path:
        sys.path.insert(0, _p)

import numpy as np
import ml_dtypes

import concourse.bass as bass
import concourse.mybir as mybir
import concourse.tile as tile
from concourse import bacc
from concourse.bass_utils import run_bass_kernel_spmd
from concourse.bass_values import RuntimeValue

B, L, CURVE_LEN = 512, 128, 300
FEAT = 3 * CURVE_LEN          # 900
FEAT_AUG = FEAT + 1           # 901 (bias row)
D_MODEL = 512
NUM_EXPERTS = 8
TOP_K = 2
EPS = 1e-9
N_CORES = 8
S = B // N_CORES              # 64 samples per core
N_KCH = 8                     # K chunks: 7 x 128 + 1 x 5
K_LAST = FEAT_AUG - 7 * 128   # 5

# matmul input dtypes: float32r streams fp32 bits at full PE rate (N>=256);
# bf16 variants halve DMA traffic at some precision cost (env-switchable for
# experiments; X_DT = stationary x dtype, W_DT = moving W dtype)
_DT_MAP = {"f32r": mybir.dt.float32r, "bf16": mybir.dt.bfloat16}
X_DT = _DT_MAP[os.environ.get("KERNEL_X_DT", "f32r")]
W_DT = _DT_MAP[os.environ.get("KERNEL_W_DT", "f32r")]

_CACHE = {}


def _build_nc():
    """Build the SPMD Bass program (routing-independent)."""
    nc = bacc.Bacc(trn_type="TRN2")
    f32 = mybir.dt.float32
    bf16 = mybir.dt.bfloat16
    i32 = mybir.dt.int32

    # x chunks 0..6: [S, 7*128, 128]; tail chunk (rows 896..900 + copy at
    # partition offset 32) as separate [S, 37, 128] tensor
    xt_h = nc.declare_dram_parameter("xt", [S, 128, 7 * 128], X_DT, isOutput=False)
    xtail_h = nc.declare_dram_parameter("xtail", [S, K_LAST, L], X_DT, isOutput=False)
    # w laid out per k-chunk: [k, part(<=128), expert, 512]
    w_h = nc.declare_dram_parameter("w", [N_KCH, 128, NUM_EXPERTS, D_MODEL], W_DT,
                                    isOutput=False)
    g_h = nc.declare_dram_parameter("g", [128, 2 * S], f32, isOutput=False)
    widx_h = nc.declare_dram_parameter("widx", [1, 2 * S], i32, isOutput=False)
    y_h = nc.declare_dram_parameter("y", [S, L, D_MODEL], bf16, isOutput=True)

    with tile.TileContext(nc) as tc:
        with (
            tc.tile_pool(name="cpool", bufs=1) as cpool,
            tc.tile_pool(name="xpool", bufs=12) as xpool,
            tc.tile_pool(name="tpool", bufs=4) as tpool,
            tc.tile_pool(name="opool", bufs=3) as opool,
            tc.tile_pool(name="pspool", bufs=8, space="PSUM") as pspool,
        ):
            # --- constants: gates, routing offsets, weights ---
            g_sb = cpool.tile([128, 2 * S], f32)
            nc.sync.dma_start(out=g_sb[:, :], in_=g_h[:, :])
            widx_sb = cpool.tile([1, 2 * S], i32)
            nc.sync.dma_start(out=widx_sb[:, :], in_=widx_h[:, :])

            # W tiles (DMAs issued after the phase-1 x preloads below)
            w_sb = []
            for k in range(N_KCH):
                wt = cpool.tile([128, NUM_EXPERTS * D_MODEL], W_DT,
                                name=f"w_sb_{k}")
                w_sb.append(wt)

            def load_w(k):
                # split each k-tile's DMA into 4 column chunks so the
                # transfers spread over many queues
                nsplit = 4
                WCOL = NUM_EXPERTS * D_MODEL // nsplit
                for c in range(nsplit):
                    nc.sync.dma_start(
                        out=w_sb[k][:, c * WCOL: (c + 1) * WCOL],
                        in_=w_h[k, :, :, :].rearrange("p e d -> p (e d)")[
                            :, c * WCOL: (c + 1) * WCOL
                        ],
                    )

            # ring of PE registers for the per-sample W-slot offsets;
            # loaded in batches of 8 (4 samples) to amortize TENSOR_LOAD cost
            NRING = 16
            wregs = [nc.tensor.alloc_register(f"widx_reg{i}") for i in range(NRING)]
            WMAX = (NUM_EXPERTS - 1) * D_MODEL

            def load_x(s):
                # host layout is partition-major: per partition one fully
                # contiguous 7*128*4B run; split in halves for 2-queue
                # parallelism; tail goes via SWDGE to keep HWDGE slots free
                x_sb = xpool.tile([128, N_KCH * 128], X_DT, tag="x",
                                  name=f"x_sb_{s}")
                H = 7 * 128 // 2   # 448
                nc.sync.dma_start(
                    out=x_sb[:, :H],
                    in_=xt_h[s, :, :H],
                )
                nc.sync.dma_start(
                    out=x_sb[:, H: 7 * 128],
                    in_=xt_h[s, :, H:],
                )
                nc.sync.dma_start(
                    out=x_sb[:K_LAST, 7 * 128: 7 * 128 + 128],
                    in_=xtail_h[s, :, :],
                )
                return x_sb

            def load_widx(s0):
                # 8 registers <- widx[2*s0 : 2*s0+8] (4 samples) in one load
                regs = [wregs[(2 * s0 + j) % NRING] for j in range(8)]
                nc.tensor.reg_load(regs, widx_sb[0:1, 2 * s0: 2 * s0 + 8])
                return [RuntimeValue(val=r, min_val=0, max_val=WMAX)
                        for r in regs]

            def mm_pair(ps, x_sb, rv, k, start, stop):
                kk = 128 if k < 7 else K_LAST
                nc.tensor.matmul(
                    ps[:, :], x_sb[:kk, k * 128: k * 128 + 128],
                    w_sb[k][:kk, bass.ds(rv, D_MODEL)],
                    start=start, stop=stop,
                )

            def combine(s, psA, psB):
                t1 = tpool.tile([128, D_MODEL], f32, tag="t", name=f"t1_{s}")
                t2 = tpool.tile([128, D_MODEL], f32, tag="t", name=f"t2_{s}")
                nc.scalar.mul(t1[:, :], psA[:, :], g_sb[:, 2 * s: 2 * s + 1])
                nc.scalar.mul(t2[:, :], psB[:, :], g_sb[:, 2 * s + 1: 2 * s + 2])
                o_sb = opool.tile([128, D_MODEL], bf16, tag="o", name=f"o_{s}")
                nc.vector.tensor_tensor(
                    o_sb[:, :], t1[:, :], t2[:, :], mybir.AluOpType.add
                )
                nc.sync.dma_start(out=y_h[s, :, :], in_=o_sb[:, :])

            # --- phase 1: first 12 samples, k-outer in 4-sample groups so
            # the PE starts as soon as w_sb[0] lands; DMA issue order is
            # interleaved to match consumption order ---
            PHASE1_GROUPS = 3
            p1_xs = [load_x(s) for s in range(4)]
            load_w(0)
            p1_xs += [load_x(4), load_x(5)]
            load_w(1)
            p1_xs += [load_x(6), load_x(7)]
            load_w(2)
            p1_xs += [load_x(8), load_x(9)]
            load_w(3)
            p1_xs += [load_x(10), load_x(11)]
            for k in range(4, N_KCH):
                load_w(k)
            for grp in range(PHASE1_GROUPS):
                s0 = grp * 4
                xs = p1_xs[s0: s0 + 4]
                rvs = load_widx(s0)
                pss = [pspool.tile([128, D_MODEL], f32, tag="ps",
                                   name=f"ps_{s0}_{j}") for j in range(8)]
                for k in range(N_KCH):
                    for j in range(4):
                        mm_pair(pss[2 * j], xs[j], rvs[2 * j], k,
                                start=(k == 0), stop=(k == N_KCH - 1))
                        mm_pair(pss[2 * j + 1], xs[j], rvs[2 * j + 1], k,
                                start=(k == 0), stop=(k == N_KCH - 1))
                for j in range(4):
                    combine(s0 + j, pss[2 * j], pss[2 * j + 1])

            # --- phase 2: steady state, sample-major ---
            for s in range(PHASE1_GROUPS * 4, S):
                x_sb = load_x(s)
                if s % 4 == 0:
                    _rvs = load_widx(s)
                    rv_cache = _rvs
                rvA = rv_cache[2 * (s % 4)]
                rvB = rv_cache[2 * (s % 4) + 1]

                psA = pspool.tile([128, D_MODEL], f32, tag="ps",
                                  name=f"psA_{s}")
                psB = pspool.tile([128, D_MODEL], f32, tag="ps",
                                  name=f"psB_{s}")
                for k in range(N_KCH):
                    mm_pair(psA, x_sb, rvA, k, start=(k == 0),
                            stop=(k == N_KCH - 1))
                    mm_pair(psB, x_sb, rvB, k, start=(k == 0),
                            stop=(k == N_KCH - 1))
                combine(s, psA, psB)

    nc.finalize()  # Bacc: reg graph-coloring + codegen passes, then freeze
    return nc


def _gates_np(logits, moe_masks):
    """Mirror reference _gates in numpy (fp32)."""
    lg = logits.astype(np.float32)
    m = lg.max(axis=1, keepdims=True)
    e = np.exp(lg - m)
    g = e / e.sum(axis=1, keepdims=True)
    g = g * (moe_masks == 1).astype(np.float32)
    # top-2, ties -> lower index first (matches jax.lax.top_k)
    top_idx = np.argsort(-g, axis=1, kind="stable")[:, :TOP_K]
    rows = np.arange(g.shape[0])[:, None]
    gsel = g[rows, top_idx]                                  # [B, 2]
    gsel = gsel / (gsel.sum(axis=1, keepdims=True) + EPS)
    return gsel.astype(np.float32), top_idx.astype(np.int32)


def _prep_inputs(cycle_curve_data, logits, moe_masks, W, b):
    gsel, top_idx = _gates_np(logits, moe_masks)

    xf = cycle_curve_data.reshape(B, L, FEAT).astype(np.float32, copy=False)
    # xt[s, p, k, l] = x[s, l, k*128 + p]  -> [B, 128, 7*128]
    xt = np.ascontiguousarray(
        xf[:, :, : 7 * 128].reshape(B, L, 7, 128).transpose(0, 3, 2, 1)
    ).reshape(B, 128, 7 * 128)
    xtail = np.empty((B, K_LAST, L), np.float32)
    xtail[:, :4, :] = xf[:, :, 7 * 128: FEAT].transpose(0, 2, 1)
    xtail[:, 4, :] = 1.0                                     # bias row

    w_aug = np.concatenate(
        [W.astype(np.float32), b.astype(np.float32)[:, None, :]], axis=1
    )                                                        # [E, 901, 512]
    w_host = np.zeros((N_KCH, 128, NUM_EXPERTS, D_MODEL), np.float32)
    for k in range(7):
        w_host[k] = w_aug[:, k * 128: (k + 1) * 128, :].transpose(1, 0, 2)
    w_host[7, :K_LAST] = w_aug[:, 7 * 128:, :].transpose(1, 0, 2)

    in_maps = []
    for c in range(N_CORES):
        sl = slice(c * S, (c + 1) * S)
        g_rep = np.broadcast_to(
            gsel[sl].reshape(1, 2 * S), (128, 2 * S)
        ).copy()
        widx = (top_idx[sl].reshape(1, 2 * S) * D_MODEL).astype(np.int32)
        x_np = np.float32 if os.environ.get("KERNEL_X_DT", "f32r") == "f32r" \
            else ml_dtypes.bfloat16
        w_np = np.float32 if os.environ.get("KERNEL_W_DT", "f32r") == "f32r" \
            else ml_dtypes.bfloat16
        in_maps.append({
            "xt": np.ascontiguousarray(xt[sl]).astype(x_np),
            "xtail": xtail[sl].astype(x_np),
            "w": w_host.astype(w_np),
            "g": g_rep,
            "widx": widx,
        })
    return in_maps


def _patch_ldw_opt():
    """Let walrus merge back-to-back LDWEIGHTS with identical stationary
    operands (the A/B expert matmuls share the x chunk): rewrite the
    hardcoded --enable-ldw-opt=false in the compile command."""
    from concourse import bass_utils as _bu
    if getattr(_bu, "_ldw_patched", False):
        return
    _orig = _bu.run_command

    def _rc(cmd, *a, **kw):
        cmd = [c.replace("--enable-ldw-opt=false", "--enable-ldw-opt=true")
               if isinstance(c, str) else c for c in cmd]
        return _orig(cmd, *a, **kw)

    _bu.run_command = _rc
    _bu._ldw_patched = True


def kernel(cycle_curve_data, logits, moe_masks, W, b):
    if os.environ.get("KERNEL_LDW_OPT", "0") == "1":
        _patch_ldw_opt()
    if "nc" not in _CACHE:
        _CACHE["nc"] = _build_nc()
    nc = _CACHE["nc"]

    in_maps = _prep_inputs(cycle_curve_data, logits, moe_masks, W, b)

    trace = bool(int(os.environ.get("KERNEL_PROFILE", "0")))
    res = run_bass_kernel_spmd(
        nc, in_maps, core_ids=list(range(N_CORES)), trace=trace
    )
    _CACHE["last_results"] = res

    out = np.empty((B, L, D_MODEL), ml_dtypes.bfloat16)
    for c in range(N_CORES):
        out[c * S: (c + 1) * S] = res.results[c]["y"]
    return out



# revision 2
# speedup vs baseline: 1.1888x; 1.1888x over previous
"""Trainium2 Bass kernel for BatteryMoEFlattenIntraCycleMoELayer.

Computation (reference):
    gates = renorm(top2(softmax(logits) * mask))          # [B, E]
    x = cycle_curve_data.reshape(B, L, 900)
    out[b] = sum_e gates[b,e] * (x[b] @ W[e] + b[e])      # -> bf16 [B, L, 512]

Strategy (v2):
  - Host: compute gates + top-2 routing (tiny). x is transposed feat-major
    into [B, 128, 1024] bf16: cols 0..895 hold chunks 0..6 (partition =
    feature-within-chunk), cols 896..1023 hold the 5-row tail (4 data rows +
    a constant-1.0 bias row, folding the bias add into the matmul via
    W_aug = [W; b]) placed at partition offset 32*(s%4) so 4 samples' tails
    can run as concurrent row-tiled matmuls.
  - Shard B across 8 cores (64 samples each). One SPMD program: routing is
    carried as *data* (per-sample W-slot element offsets, read into PE
    registers at runtime -> dynamic access patterns on the matmul moving
    operand).
  - Device per sample: DVE scales x by the two gates (xA = g0*x, xB = g1*x,
    bf16) so BOTH experts' matmuls accumulate into ONE PSUM bank
    (out = xA @ W[e0] + xB @ W[e1], fp32 accumulate); a single ACT copy
    downcasts PSUM -> bf16 SBUF. bf16 operands stream at the same PE rate
    as f32r but get fast standalone LDWEIGHTS (FWL) that the PE pulls ahead
    of in-flight matmuls, and half the DMA traffic.
  - The K=5 tail chunk of 4 samples runs as 4 concurrent row-tiled matmuls
    (tile_position=(32j,0), one PSUM bank each), recovering most of the
    2x512-cycle padding waste per sample.
  - Phase 1 runs k-outer across the first 8 samples (one PSUM bank each) so
    the PE starts ~3us in and stays dense while the 8 MB of bf16 weights
    stream in; phase 2 is sample-major with pool-paced DMA prefetch.
"""

import os
import sys

for _p in ("/opt/trn_rl_repo", "/root/.axon_site/_ro/trn_rl_repo"):
    if os.path.isdir(_p) and _p not in sys.path:
        sys.path.insert(0, _p)

import numpy as np
import ml_dtypes

import concourse.bass as bass
import concourse.mybir as mybir
import concourse.tile as tile
from concourse import bacc
from concourse.bass_utils import run_bass_kernel_spmd
from concourse.bass_values import RuntimeValue

B, L, CURVE_LEN = 512, 128, 300
FEAT = 3 * CURVE_LEN          # 900
FEAT_AUG = FEAT + 1           # 901 (bias row)
D_MODEL = 512
NUM_EXPERTS = 8
TOP_K = 2
EPS = 1e-9
N_CORES = 8
S = B // N_CORES              # 64 samples per core
N_KCH = 8                     # K chunks: 7 x 128 + 1 x 5 (tail incl bias)
K_LAST = FEAT_AUG - 7 * 128   # 5
XCOLS = 1024                  # 7*128 main cols + 128 tail cols
WMAX = (NUM_EXPERTS - 1) * D_MODEL

_CACHE = {}


def _build_nc():
    """Build the SPMD Bass program (routing-independent)."""
    nc = bacc.Bacc(trn_type="TRN2")
    f32 = mybir.dt.float32
    bf16 = mybir.dt.bfloat16
    i32 = mybir.dt.int32

    # x: [S, 128, 1024] bf16, feat-major; tail at partition 32*(s%4),
    # cols 896..1023 (host zero-pads the rest of the tail cols)
    x_h = nc.declare_dram_parameter("x", [S, 128, XCOLS], bf16, isOutput=False)
    # w laid out per k-chunk: [k, part(<=128), expert, 512]; k=7 holds the
    # 5 tail rows replicated at partition offsets 0/32/64/96
    w_h = nc.declare_dram_parameter("w", [N_KCH, 128, NUM_EXPERTS, D_MODEL],
                                    bf16, isOutput=False)
    g_h = nc.declare_dram_parameter("g", [128, 2 * S], f32, isOutput=False)
    widx_h = nc.declare_dram_parameter("widx", [1, 2 * S], i32, isOutput=False)
    y_h = nc.declare_dram_parameter("y", [S, L, D_MODEL], bf16, isOutput=True)

    with tile.TileContext(nc) as tc:
        with (
            tc.tile_pool(name="cpool", bufs=1) as cpool,
            tc.tile_pool(name="xpool", bufs=10) as xpool,
            tc.tile_pool(name="spool", bufs=10) as spool,
            tc.tile_pool(name="opool", bufs=4) as opool,
            tc.tile_pool(name="pspool", bufs=8, space="PSUM") as pspool,
        ):
            # --- constants: gates, routing offsets, weights ---
            g_sb = cpool.tile([128, 2 * S], f32)
            nc.sync.dma_start(out=g_sb[:, :], in_=g_h[:, :])
            widx_sb = cpool.tile([1, 2 * S], i32)
            nc.sync.dma_start(out=widx_sb[:, :], in_=widx_h[:, :])

            w_sb = []
            for k in range(N_KCH):
                wt = cpool.tile([128, NUM_EXPERTS * D_MODEL], bf16,
                                name=f"w_sb_{k}")
                w_sb.append(wt)

            def load_w(k):
                # split each k-tile's DMA into 2 column halves so transfers
                # spread over queues
                WCOL = NUM_EXPERTS * D_MODEL // 2
                for c in range(2):
                    nc.sync.dma_start(
                        out=w_sb[k][:, c * WCOL: (c + 1) * WCOL],
                        in_=w_h[k, :, :, :].rearrange("p e d -> p (e d)")[
                            :, c * WCOL: (c + 1) * WCOL
                        ],
                    )

            # ring of PE registers for the per-sample W-slot offsets;
            # 16 regs (8 samples) per TENSOR_LOAD, double-buffered ring of 32
            NRING = 32
            wregs = [nc.tensor.alloc_register(f"widx_reg{i}")
                     for i in range(NRING)]

            def load_widx_span(s0):
                # 16 registers <- widx[2*s0 : 2*s0+16] (8 samples)
                regs = [wregs[(2 * s0 + j) % NRING] for j in range(16)]
                nc.tensor.reg_load(regs, widx_sb[0:1, 2 * s0: 2 * s0 + 16])

            def rv_of(s):
                a = RuntimeValue(val=wregs[(2 * s) % NRING], min_val=0,
                                 max_val=WMAX)
                b = RuntimeValue(val=wregs[(2 * s + 1) % NRING], min_val=0,
                                 max_val=WMAX)
                return a, b

            def load_x(s, split=True):
                x_sb = xpool.tile([128, XCOLS], bf16, tag="x",
                                  name=f"x_sb_{s}")
                if split:
                    H = XCOLS // 2
                    nc.sync.dma_start(out=x_sb[:, :H], in_=x_h[s, :, :H])
                    nc.sync.dma_start(out=x_sb[:, H:], in_=x_h[s, :, H:])
                else:
                    nc.sync.dma_start(out=x_sb[:, :], in_=x_h[s, :, :])
                return x_sb

            def scale_x(s, x_sb):
                # xs[:, 0:1024] = g0 * x ; xs[:, 1024:2048] = g1 * x
                # split per half so phase-1 k=0..3 can start after half 0
                xs = spool.tile([128, 2 * XCOLS], bf16, tag="xs",
                                name=f"xs_{s}")
                H = XCOLS // 2
                gA = g_sb[:, 2 * s: 2 * s + 1]
                gB = g_sb[:, 2 * s + 1: 2 * s + 2]
                nc.vector.tensor_scalar_mul(xs[:, 0:H], x_sb[:, 0:H], gA)
                nc.vector.tensor_scalar_mul(
                    xs[:, XCOLS: XCOLS + H], x_sb[:, 0:H], gB)
                nc.vector.tensor_scalar_mul(xs[:, H: XCOLS], x_sb[:, H:], gA)
                nc.vector.tensor_scalar_mul(
                    xs[:, XCOLS + H: 2 * XCOLS], x_sb[:, H:], gB)
                return xs

            def mm_main(ps, xs, rv, half, k, start):
                # half 0 = expert A (cols 0:1024), half 1 = expert B
                base = half * XCOLS
                nc.tensor.matmul(
                    ps[:, :],
                    xs[:, base + k * 128: base + (k + 1) * 128],
                    w_sb[k][:, bass.ds(rv, D_MODEL)],
                    start=start, stop=False,
                )

            def mm_tail(ps, xs, rv, half, j, stop):
                # K=5 tail at partition offset 32j, concurrent row-tiled
                base = half * XCOLS
                p0 = 32 * j
                nc.tensor.matmul(
                    ps[:, :],
                    xs[p0: p0 + K_LAST, base + 7 * 128: base + XCOLS],
                    w_sb[7][p0: p0 + K_LAST, bass.ds(rv, D_MODEL)],
                    start=False, stop=stop,
                    tile_position=(p0, 0),
                )

            def combine(s, ps):
                o_sb = opool.tile([128, D_MODEL], bf16, tag="o",
                                  name=f"o_{s}")
                nc.scalar.copy(o_sb[:, :], ps[:, :])
                nc.sync.dma_start(out=y_h[s, :, :], in_=o_sb[:, :])

            # --- phase 1: first 8 samples, k-outer so the PE starts as soon
            # as w_sb[0] + the first x halves land ---
            P1 = 8
            load_w(0)
            p1_x = [load_x(s) for s in range(P1)]
            load_w(1)
            p1_xs = [scale_x(s, p1_x[s]) for s in range(P1)]
            for k in range(2, N_KCH):
                load_w(k)
            load_widx_span(0)
            p1_ps = [pspool.tile([128, D_MODEL], f32, tag="ps",
                                 name=f"ps_{s}") for s in range(P1)]
            p1_rv = [rv_of(s) for s in range(P1)]
            for k in range(7):
                for s in range(P1):
                    rvA, rvB = p1_rv[s]
                    mm_main(p1_ps[s], p1_xs[s], rvA, 0, k, start=(k == 0))
                    mm_main(p1_ps[s], p1_xs[s], rvB, 1, k, start=False)
            for grp in range(2):
                for j in range(4):
                    s = grp * 4 + j
                    mm_tail(p1_ps[s], p1_xs[s], p1_rv[s][0], 0, j, stop=False)
                for j in range(4):
                    s = grp * 4 + j
                    mm_tail(p1_ps[s], p1_xs[s], p1_rv[s][1], 1, j, stop=True)
            for s in range(P1):
                combine(s, p1_ps[s])

            # --- phase 2: steady state, groups of 4 samples ---
            for grp in range(2, S // 4):
                xs_g, ps_g, rv_g = [], [], []
                for j in range(4):
                    s = grp * 4 + j
                    if s % 8 == 0:
                        load_widx_span(s)
                    x_sb = load_x(s)
                    xs_g.append(scale_x(s, x_sb))
                    ps_g.append(pspool.tile([128, D_MODEL], f32, tag="ps",
                                            name=f"ps_{s}"))
                    rv_g.append(rv_of(s))
                for j in range(4):
                    rvA, rvB = rv_g[j]
                    for k in range(7):
                        mm_main(ps_g[j], xs_g[j], rvA, 0, k, start=(k == 0))
                    for k in range(7):
                        mm_main(ps_g[j], xs_g[j], rvB, 1, k, start=False)
                for j in range(4):
                    mm_tail(ps_g[j], xs_g[j], rv_g[j][0], 0, j, stop=False)
                for j in range(4):
                    mm_tail(ps_g[j], xs_g[j], rv_g[j][1], 1, j, stop=True)
                for j in range(4):
                    combine(grp * 4 + j, ps_g[j])

    nc.finalize()  # Bacc: reg graph-coloring + codegen passes, then freeze
    return nc


def _gates_np(logits, moe_masks):
    """Mirror reference _gates in numpy (fp32)."""
    lg = logits.astype(np.float32)
    m = lg.max(axis=1, keepdims=True)
    e = np.exp(lg - m)
    g = e / e.sum(axis=1, keepdims=True)
    g = g * (moe_masks == 1).astype(np.float32)
    # top-2, ties -> lower index first (matches jax.lax.top_k)
    top_idx = np.argsort(-g, axis=1, kind="stable")[:, :TOP_K]
    rows = np.arange(g.shape[0])[:, None]
    gsel = g[rows, top_idx]                                  # [B, 2]
    gsel = gsel / (gsel.sum(axis=1, keepdims=True) + EPS)
    return gsel.astype(np.float32), top_idx.astype(np.int32)


def _prep_inputs(cycle_curve_data, logits, moe_masks, W, b):
    gsel, top_idx = _gates_np(logits, moe_masks)

    xf = cycle_curve_data.reshape(B, L, FEAT).astype(np.float32, copy=False)
    # x_host[s, p, k*128 + l] = x[s, l, k*128 + p] for chunks 0..6;
    # tail rows 896..899 + bias row at partitions 32*(s%4)..+4, cols 896..1023
    x_host = np.zeros((B, 128, XCOLS), np.float32)
    x_host[:, :, : 7 * 128] = np.ascontiguousarray(
        xf[:, :, : 7 * 128].reshape(B, L, 7, 128).transpose(0, 3, 2, 1)
    ).reshape(B, 128, 7 * 128)
    tail = np.empty((B, K_LAST, L), np.float32)
    tail[:, :4, :] = xf[:, :, 7 * 128: FEAT].transpose(0, 2, 1)
    tail[:, 4, :] = 1.0                                      # bias row
    for j in range(4):
        x_host[j::4, 32 * j: 32 * j + K_LAST, 7 * 128:] = tail[j::4]
    x_host = x_host.astype(ml_dtypes.bfloat16)

    w_aug = np.concatenate(
        [W.astype(np.float32), b.astype(np.float32)[:, None, :]], axis=1
    )                                                        # [E, 901, 512]
    w_host = np.zeros((N_KCH, 128, NUM_EXPERTS, D_MODEL), np.float32)
    for k in range(7):
        w_host[k] = w_aug[:, k * 128: (k + 1) * 128, :].transpose(1, 0, 2)
    for j in range(4):
        w_host[7, 32 * j: 32 * j + K_LAST] = \
            w_aug[:, 7 * 128:, :].transpose(1, 0, 2)
    w_host = w_host.astype(ml_dtypes.bfloat16)

    in_maps = []
    for c in range(N_CORES):
        sl = slice(c * S, (c + 1) * S)
        g_rep = np.broadcast_to(
            gsel[sl].reshape(1, 2 * S), (128, 2 * S)
        ).copy()
        widx = (top_idx[sl].reshape(1, 2 * S) * D_MODEL).astype(np.int32)
        in_maps.append({
            "x": np.ascontiguousarray(x_host[sl]),
            "w": w_host,
            "g": g_rep,
            "widx": widx,
        })
    return in_maps


def kernel(cycle_curve_data, logits, moe_masks, W, b):
    if "nc" not in _CACHE:
        _CACHE["nc"] = _build_nc()
    nc = _CACHE["nc"]

    in_maps = _prep_inputs(cycle_curve_data, logits, moe_masks, W, b)

    trace = bool(int(os.environ.get("KERNEL_PROFILE", "0")))
    res = run_bass_kernel_spmd(
        nc, in_maps, core_ids=list(range(N_CORES)), trace=trace
    )
    _CACHE["last_results"] = res

    out = np.empty((B, L, D_MODEL), ml_dtypes.bfloat16)
    for c in range(N_CORES):
        out[c * S: (c + 1) * S] = res.results[c]["y"]
    return out


# revision 8
# speedup vs baseline: 1.3432x; 1.1299x over previous
"""Trainium2 Bass kernel for BatteryMoEFlattenIntraCycleMoELayer.

Computation (reference):
    gates = renorm(top2(softmax(logits) * mask))          # [B, E]
    x = cycle_curve_data.reshape(B, L, 900)
    out[b] = sum_e gates[b,e] * (x[b] @ W[e] + b[e])      # -> bf16 [B, L, 512]

Strategy (v3):
  - Host: gates + top-2 routing (tiny). x transposed feat-major into
    [B, 128, 1024] bf16: cols 0..895 = chunks 0..6 (partition =
    feature-within-chunk), cols 896..1023 = the 5-row tail (4 data rows +
    constant-1.0 bias row; bias folds into the matmul via W_aug = [W; b])
    placed at a per-sample partition offset so tails of neighboring samples
    run as concurrent row-tiled matmuls.
  - Shard B across 8 cores (64 samples each). One SPMD program: routing is
    data (per-sample W-slot offsets in PE registers, dynamic APs on the
    moving operand). All 128 offsets are register-resident: 8 TENSOR_LOADs
    interleaved into the DMA-bound phase 1, so steady state never stalls.
  - Phase 1 (samples 0..7): DVE pre-scales x by the gates (xA=g0*x, xB=g1*x)
    so both experts accumulate into ONE PSUM bank per sample; k-outer over
    8 samples paces the PE exactly against the streaming weight DMAs
    (w[k] chunk feeds 16 N=512 matmuls). Single ACT copy per sample.
  - Phase 2 (steady): pairs of samples, UNSCALED x stationary shared by the
    two expert matmuls (one LDWEIGHTS per 2 matmuls) alternating two PSUM
    banks (avoids the same-bank accumulation stall); gates applied post-hoc
    on ACT (2 muls) + DVE add, as combine. Tails of the 2 samples run as
    row-tiled concurrent matmuls at partition offsets 0/32.
  - bf16 operands: same PE stream rate as f32r, half the DMA, fast FWL
    LDWEIGHTS the PE pulls ahead of in-flight matmuls.
"""

import os
import sys

for _p in ("/opt/trn_rl_repo", "/root/.axon_site/_ro/trn_rl_repo"):
    if os.path.isdir(_p) and _p not in sys.path:
        sys.path.insert(0, _p)

import numpy as np
import ml_dtypes

import concourse.bass as bass
import concourse.mybir as mybir
import concourse.tile as tile
from concourse import bacc
from concourse.bass_utils import run_bass_kernel_spmd
from concourse.bass_values import RuntimeValue

B, L, CURVE_LEN = 512, 128, 300
FEAT = 3 * CURVE_LEN          # 900
FEAT_AUG = FEAT + 1           # 901 (bias row)
D_MODEL = 512
NUM_EXPERTS = 8
TOP_K = 2
EPS = 1e-9
N_CORES = 8
S = B // N_CORES              # 64 samples per core
N_KCH = 8                     # K chunks: 7 x 128 + 1 x 5 (tail incl bias)
K_LAST = FEAT_AUG - 7 * 128   # 5
XCOLS = 1024                  # 7*128 main cols + 128 tail cols
WMAX = (NUM_EXPERTS - 1) * D_MODEL
P1 = 8                        # phase-1 sample count

_CACHE = {}


def _tail_poff(s):
    """Partition offset of sample s's tail rows (row-group for tiling)."""
    return 32 * (s % 4) if s < P1 else 32 * (s % 2)


def _build_nc():
    """Build the SPMD Bass program (routing-independent)."""
    nc = bacc.Bacc(trn_type="TRN2")
    f32 = mybir.dt.float32
    bf16 = mybir.dt.bfloat16
    i32 = mybir.dt.int32

    x_h = nc.declare_dram_parameter("x", [S, 128, XCOLS], bf16, isOutput=False)
    # w per k-chunk: [k, part(<=128), expert, 512]; k=7 holds the 5 tail
    # rows replicated at partition offsets 0/32/64/96
    w_h = nc.declare_dram_parameter("w", [N_KCH, 128, NUM_EXPERTS, D_MODEL],
                                    bf16, isOutput=False)
    g_h = nc.declare_dram_parameter("g", [128, 2 * S], f32, isOutput=False)
    widx_h = nc.declare_dram_parameter("widx", [1, 2 * S], i32, isOutput=False)
    y_h = nc.declare_dram_parameter("y", [S, L, D_MODEL], bf16, isOutput=True)

    with tile.TileContext(nc) as tc:
        with (
            tc.tile_pool(name="cpool", bufs=1) as cpool,
            tc.tile_pool(name="xpool", bufs=10) as xpool,
            tc.tile_pool(name="spool", bufs=8) as spool,
            tc.tile_pool(name="tpool", bufs=4) as tpool,
            tc.tile_pool(name="opool", bufs=4) as opool,
            tc.tile_pool(name="pspool", bufs=8, space="PSUM") as pspool,
        ):
            # --- constants ---
            g_sb = cpool.tile([128, 2 * S], f32)
            nc.sync.dma_start(out=g_sb[:, :], in_=g_h[:, :])
            widx_sb = cpool.tile([1, 2 * S], i32)
            nc.sync.dma_start(out=widx_sb[:, :], in_=widx_h[:, :])

            w_sb = [cpool.tile([128, NUM_EXPERTS * D_MODEL], bf16,
                               name=f"w_sb_{k}") for k in range(N_KCH)]

            def load_w(k, nsplit):
                WCOL = NUM_EXPERTS * D_MODEL // nsplit
                for c in range(nsplit):
                    nc.sync.dma_start(
                        out=w_sb[k][:, c * WCOL: (c + 1) * WCOL],
                        in_=w_h[k, :, :, :].rearrange("p e d -> p (e d)")[
                            :, c * WCOL: (c + 1) * WCOL
                        ],
                    )

            def load_w_tail():
                # only the 4 replicated 5-row strips of k=7 carry data
                for j in range(4):
                    p0 = 32 * j
                    nc.sync.dma_start(
                        out=w_sb[7][p0: p0 + K_LAST, :],
                        in_=w_h[7, p0: p0 + K_LAST, :, :].rearrange(
                            "p e d -> p (e d)"),
                    )

            # all 128 per-sample W-slot offsets stay register-resident
            wregs = [nc.tensor.alloc_register(f"widx_reg{i}")
                     for i in range(2 * S)]

            def load_widx_span(h):
                # 16 registers <- widx[16h : 16h+16] (8 samples)
                nc.tensor.reg_load(wregs[16 * h: 16 * h + 16],
                                   widx_sb[0:1, 16 * h: 16 * h + 16])

            def rv_of(s):
                a = RuntimeValue(val=wregs[2 * s], min_val=0, max_val=WMAX)
                b = RuntimeValue(val=wregs[2 * s + 1], min_val=0,
                                 max_val=WMAX)
                return a, b

            def load_x(s):
                x_sb = xpool.tile([128, XCOLS], bf16, tag="x",
                                  name=f"x_sb_{s}")
                H = XCOLS // 2
                nc.sync.dma_start(out=x_sb[:, :H], in_=x_h[s, :, :H])
                nc.sync.dma_start(out=x_sb[:, H:], in_=x_h[s, :, H:])
                return x_sb

            def scale_x(s, x_sb):
                # xs[:, 0:1024] = g0 * x ; xs[:, 1024:2048] = g1 * x
                xs = spool.tile([128, 2 * XCOLS], bf16, tag="xs",
                                name=f"xs_{s}")
                H = XCOLS // 2
                gA = g_sb[:, 2 * s: 2 * s + 1]
                gB = g_sb[:, 2 * s + 1: 2 * s + 2]
                nc.vector.tensor_scalar_mul(xs[:, 0:H], x_sb[:, 0:H], gA)
                nc.vector.tensor_scalar_mul(
                    xs[:, XCOLS: XCOLS + H], x_sb[:, 0:H], gB)
                nc.vector.tensor_scalar_mul(xs[:, H: XCOLS], x_sb[:, H:], gA)
                nc.vector.tensor_scalar_mul(
                    xs[:, XCOLS + H: 2 * XCOLS], x_sb[:, H:], gB)
                return xs

            def mm_tail(ps, xt, rv, p0, stop):
                # K=5 tail at partition offset p0, row-tiled
                nc.tensor.matmul(
                    ps[:, :],
                    xt[p0: p0 + K_LAST, 7 * 128: XCOLS],
                    w_sb[7][p0: p0 + K_LAST, bass.ds(rv, D_MODEL)],
                    start=False, stop=stop,
                    tile_position=(p0, 0),
                )

            # ---------- phase 1: samples 0..7, scaled one-bank k-outer ----
            load_w_tail()
            load_w(0, 8)
            p1_x = [load_x(s) for s in range(2)]
            load_w(1, 4)
            p1_x += [load_x(s) for s in range(2, 4)]
            load_w(2, 2)
            p1_x += [load_x(s) for s in range(4, 8)]
            for k in range(3, 7):
                load_w(k, 2)
            p1_xs = [scale_x(s, p1_x[s]) for s in range(P1)]
            load_widx_span(0)
            p1_ps = [pspool.tile([128, D_MODEL], f32, tag="ps",
                                 name=f"ps_{s}") for s in range(P1)]
            p1_rv = [rv_of(s) for s in range(P1)]
            for k in range(7):
                for s in range(P1):       # expert-A sweep (banks cycle 0..7)
                    nc.tensor.matmul(
                        p1_ps[s][:, :],
                        p1_xs[s][:, k * 128: (k + 1) * 128],
                        w_sb[k][:, bass.ds(p1_rv[s][0], D_MODEL)],
                        start=(k == 0), stop=False,
                    )
                for s in range(P1):       # expert-B sweep
                    nc.tensor.matmul(
                        p1_ps[s][:, :],
                        p1_xs[s][:, XCOLS + k * 128: XCOLS + (k + 1) * 128],
                        w_sb[k][:, bass.ds(p1_rv[s][1], D_MODEL)],
                        start=False, stop=False,
                    )
                load_widx_span(k + 1)     # hide reg loads in DMA-bound phase
            for grp in range(2):
                for j in range(4):
                    s = grp * 4 + j
                    mm_tail(p1_ps[s], p1_xs[s][:, 0: XCOLS],
                            p1_rv[s][0], 32 * j, stop=False)
                for j in range(4):
                    s = grp * 4 + j
                    mm_tail(p1_ps[s], p1_xs[s][:, XCOLS: 2 * XCOLS],
                            p1_rv[s][1], 32 * j, stop=True)
            for s in range(P1):
                o_sb = opool.tile([128, D_MODEL], bf16, tag="o",
                                  name=f"o_{s}")
                # alternate engines so PSUM banks free 2-at-a-time for
                # the phase-2 pipeline start
                if s % 2 == 0:
                    nc.scalar.copy(o_sb[:, :], p1_ps[s][:, :])
                else:
                    nc.vector.tensor_copy(o_sb[:, :], p1_ps[s][:, :])
                nc.sync.dma_start(out=y_h[s, :, :], in_=o_sb[:, :])

            # ---------- phase 2: sample pairs, shared-stationary 2-bank ----
            for pair in range(P1 // 2, S // 2):
                s0 = 2 * pair
                xp, psA, psB, rvp = [], [], [], []
                for j in range(2):
                    s = s0 + j
                    xp.append(load_x(s))
                    psA.append(pspool.tile([128, D_MODEL], f32, tag="ps",
                                           name=f"psA_{s}"))
                    psB.append(pspool.tile([128, D_MODEL], f32, tag="ps",
                                           name=f"psB_{s}"))
                    rvp.append(rv_of(s))
                for j in range(2):
                    rvA, rvB = rvp[j]
                    for k in range(7):
                        # shared stationary x chunk, banks alternate A/B
                        nc.tensor.matmul(
                            psA[j][:, :],
                            xp[j][:, k * 128: (k + 1) * 128],
                            w_sb[k][:, bass.ds(rvA, D_MODEL)],
                            start=(k == 0), stop=False,
                        )
                        nc.tensor.matmul(
                            psB[j][:, :],
                            xp[j][:, k * 128: (k + 1) * 128],
                            w_sb[k][:, bass.ds(rvB, D_MODEL)],
                            start=(k == 0), stop=False,
                        )
                # tails: row groups 0 / 32; A-sweep then B-sweep so the two
                # row groups overlap (matmul starts are pc-monotone)
                for j in range(2):
                    mm_tail(psA[j], xp[j], rvp[j][0], 32 * j, stop=True)
                for j in range(2):
                    mm_tail(psB[j], xp[j], rvp[j][1], 32 * j, stop=True)
                for j in range(2):
                    s = s0 + j
                    t1 = tpool.tile([128, D_MODEL], f32, tag="t",
                                    name=f"t1_{s}")
                    t2 = tpool.tile([128, D_MODEL], f32, tag="t",
                                    name=f"t2_{s}")
                    nc.scalar.mul(t1[:, :], psA[j][:, :],
                                  g_sb[:, 2 * s: 2 * s + 1])
                    nc.scalar.mul(t2[:, :], psB[j][:, :],
                                  g_sb[:, 2 * s + 1: 2 * s + 2])
                    o_sb = opool.tile([128, D_MODEL], bf16, tag="o",
                                      name=f"o_{s}")
                    nc.vector.tensor_tensor(
                        o_sb[:, :], t1[:, :], t2[:, :], mybir.AluOpType.add
                    )
                    nc.sync.dma_start(out=y_h[s, :, :], in_=o_sb[:, :])

    nc.finalize()
    return nc


def _gates_np(logits, moe_masks):
    """Mirror reference _gates in numpy (fp32)."""
    lg = logits.astype(np.float32)
    m = lg.max(axis=1, keepdims=True)
    e = np.exp(lg - m)
    g = e / e.sum(axis=1, keepdims=True)
    g = g * (moe_masks == 1).astype(np.float32)
    # top-2, ties -> lower index first (matches jax.lax.top_k)
    top_idx = np.argsort(-g, axis=1, kind="stable")[:, :TOP_K]
    rows = np.arange(g.shape[0])[:, None]
    gsel = g[rows, top_idx]                                  # [B, 2]
    gsel = gsel / (gsel.sum(axis=1, keepdims=True) + EPS)
    return gsel.astype(np.float32), top_idx.astype(np.int32)


def _prep_inputs(cycle_curve_data, logits, moe_masks, W, b):
    gsel, top_idx = _gates_np(logits, moe_masks)

    xf = cycle_curve_data.reshape(B, L, FEAT).astype(np.float32, copy=False)
    # x_host[s, p, k*128 + l] = x[s, l, k*128 + p] for chunks 0..6;
    # tail rows 896..899 + bias row at the per-sample partition offset
    x_host = np.zeros((B, 128, XCOLS), np.float32)
    x_host[:, :, : 7 * 128] = np.ascontiguousarray(
        xf[:, :, : 7 * 128].reshape(B, L, 7, 128).transpose(0, 3, 2, 1)
    ).reshape(B, 128, 7 * 128)
    tail = np.empty((B, K_LAST, L), np.float32)
    tail[:, :4, :] = xf[:, :, 7 * 128: FEAT].transpose(0, 2, 1)
    tail[:, 4, :] = 1.0                                      # bias row
    poff = np.array([_tail_poff(s % S) for s in range(B)])
    for p0 in (0, 32, 64, 96):
        m = poff == p0
        x_host[m, p0: p0 + K_LAST, 7 * 128:] = tail[m]
    x_host = x_host.astype(ml_dtypes.bfloat16)

    w_aug = np.concatenate(
        [W.astype(np.float32), b.astype(np.float32)[:, None, :]], axis=1
    )                                                        # [E, 901, 512]
    w_host = np.zeros((N_KCH, 128, NUM_EXPERTS, D_MODEL), np.float32)
    for k in range(7):
        w_host[k] = w_aug[:, k * 128: (k + 1) * 128, :].transpose(1, 0, 2)
    for j in range(4):
        w_host[7, 32 * j: 32 * j + K_LAST] = \
            w_aug[:, 7 * 128:, :].transpose(1, 0, 2)
    w_host = w_host.astype(ml_dtypes.bfloat16)

    in_maps = []
    for c in range(N_CORES):
        sl = slice(c * S, (c + 1) * S)
        g_rep = np.broadcast_to(
            gsel[sl].reshape(1, 2 * S), (128, 2 * S)
        ).copy()
        widx = (top_idx[sl].reshape(1, 2 * S) * D_MODEL).astype(np.int32)
        in_maps.append({
            "x": np.ascontiguousarray(x_host[sl]),
            "w": w_host,
            "g": g_rep,
            "widx": widx,
        })
    return in_maps


def kernel(cycle_curve_data, logits, moe_masks, W, b):
    if "nc" not in _CACHE:
        _CACHE["nc"] = _build_nc()
    nc = _CACHE["nc"]

    in_maps = _prep_inputs(cycle_curve_data, logits, moe_masks, W, b)

    trace = bool(int(os.environ.get("KERNEL_PROFILE", "0")))
    res = run_bass_kernel_spmd(
        nc, in_maps, core_ids=list(range(N_CORES)), trace=trace
    )
    _CACHE["last_results"] = res

    out = np.empty((B, L, D_MODEL), ml_dtypes.bfloat16)
    for c in range(N_CORES):
        out[c * S: (c + 1) * S] = res.results[c]["y"]
    return out


# revision 9
# speedup vs baseline: 1.3626x; 1.0144x over previous
"""Trainium2 Bass kernel for BatteryMoEFlattenIntraCycleMoELayer.

Computation (reference):
    gates = renorm(top2(softmax(logits) * mask))          # [B, E]
    x = cycle_curve_data.reshape(B, L, 900)
    out[b] = sum_e gates[b,e] * (x[b] @ W[e] + b[e])      # -> bf16 [B, L, 512]

Strategy (v4):
  - Host: gates + top-2 routing (tiny). x transposed feat-major into
    [B, 128, 1024] bf16: cols 0..895 = chunks 0..6 (partition =
    feature-within-chunk), cols 896..1023 = the 5-row tail (4 data rows +
    constant-1.0 bias row; bias folds into the matmul via W_aug = [W; b])
    at a per-sample partition offset so neighboring samples' tails run as
    concurrent row-tiled matmuls.
  - Shard B across 8 cores (64 samples each). One SPMD program: routing is
    data (per-sample W-slot offsets in PE registers, dynamic APs on the
    moving operand). All 128 offsets are register-resident: 8 TENSOR_LOADs
    tucked into the DMA-bound phase 1, so steady state never stalls.
  - Dummy fp32 matmuls on the gates tile warm the PE HAM clock gate during
    the initial DMA wait (otherwise the first ~3.4us of real matmuls run at
    1.2 GHz instead of 2.4).
  - Phase 1 (samples 0..7): DVE pre-scales x by the gates (xA=g0*x, xB=g1*x)
    so both experts accumulate into ONE PSUM bank per sample; k-outer over
    8 samples paces the PE against the streaming weight DMAs. One
    ACT-or-DVE copy per sample.
  - Phase 2 (steady): pairs of samples, UNSCALED x stationary shared by the
    two expert matmuls (one LDWEIGHTS per 2 matmuls) alternating two PSUM
    banks (avoids the same-bank accumulation stall); combine = ACT mul
    (t=g1*psB) + DVE scalar_tensor_tensor (o = g0*psA + t, bf16). Tails of
    the 2 samples run row-tiled at partition offsets 0/32.
  - DMA discipline: each dma_start costs ~650ns of Sync-queue issue time,
    so transfers are few and big, issued in consumption order; y outputs
    go out via the otherwise-idle GpSimd (SWDGE) queue.
"""

import os
import sys

for _p in ("/opt/trn_rl_repo", "/root/.axon_site/_ro/trn_rl_repo"):
    if os.path.isdir(_p) and _p not in sys.path:
        sys.path.insert(0, _p)

import numpy as np
import ml_dtypes

import concourse.bass as bass
import concourse.mybir as mybir
import concourse.tile as tile
from concourse import bacc
from concourse.bass_utils import run_bass_kernel_spmd
from concourse.bass_values import RuntimeValue

B, L, CURVE_LEN = 512, 128, 300
FEAT = 3 * CURVE_LEN          # 900
FEAT_AUG = FEAT + 1           # 901 (bias row)
D_MODEL = 512
NUM_EXPERTS = 8
TOP_K = 2
EPS = 1e-9
N_CORES = 8
S = B // N_CORES              # 64 samples per core
N_KCH = 8                     # K chunks: 7 x 128 + 1 x 5 (tail incl bias)
K_LAST = FEAT_AUG - 7 * 128   # 5
XCOLS = 1024                  # 7*128 main cols + 128 tail cols
WMAX = (NUM_EXPERTS - 1) * D_MODEL
P1 = 8                        # phase-1 sample count

_CACHE = {}


def _tail_poff(s):
    """Partition offset of sample s's tail rows (row-group for tiling)."""
    return 32 * (s % 4) if s < P1 else 32 * (s % 2)


def _build_nc():
    """Build the SPMD Bass program (routing-independent)."""
    nc = bacc.Bacc(trn_type="TRN2")
    f32 = mybir.dt.float32
    bf16 = mybir.dt.bfloat16
    i32 = mybir.dt.int32

    x_h = nc.declare_dram_parameter("x", [S, 128, XCOLS], bf16, isOutput=False)
    # w per k-chunk: [k, part(<=128), expert, 512]; k=7 holds the 5 tail
    # rows replicated at partition offsets 0/32/64/96
    w_h = nc.declare_dram_parameter("w", [N_KCH, 128, NUM_EXPERTS, D_MODEL],
                                    bf16, isOutput=False)
    g_h = nc.declare_dram_parameter("g", [128, 2 * S], f32, isOutput=False)
    widx_h = nc.declare_dram_parameter("widx", [1, 2 * S], i32, isOutput=False)
    y_h = nc.declare_dram_parameter("y", [S, L, D_MODEL], bf16, isOutput=True)

    with tile.TileContext(nc) as tc:
        with (
            tc.tile_pool(name="cpool", bufs=1) as cpool,
            tc.tile_pool(name="xpool", bufs=12) as xpool,
            tc.tile_pool(name="spool", bufs=8) as spool,
            tc.tile_pool(name="tpool", bufs=4) as tpool,
            tc.tile_pool(name="opool", bufs=4) as opool,
            tc.tile_pool(name="pspool", bufs=8, space="PSUM") as pspool,
        ):
            # --- constants; issue order == consumption order ---
            widx_sb = cpool.tile([1, 2 * S], i32)
            nc.sync.dma_start(out=widx_sb[:, :], in_=widx_h[:, :])
            g_sb = cpool.tile([128, 2 * S], f32)
            nc.sync.dma_start(out=g_sb[:, :], in_=g_h[:, :])

            w_sb = [cpool.tile([128, NUM_EXPERTS * D_MODEL], bf16,
                               name=f"w_sb_{k}") for k in range(N_KCH)]

            def load_w(k):
                nc.sync.dma_start(
                    out=w_sb[k][:, :],
                    in_=w_h[k, :, :, :].rearrange("p e d -> p (e d)"),
                )

            def load_w_tail():
                # only the 4 replicated 5-row strips of k=7 carry data
                for j in range(4):
                    p0 = 32 * j
                    nc.sync.dma_start(
                        out=w_sb[7][p0: p0 + K_LAST, :],
                        in_=w_h[7, p0: p0 + K_LAST, :, :].rearrange(
                            "p e d -> p (e d)"),
                    )

            # all 128 per-sample W-slot offsets stay register-resident
            wregs = [nc.tensor.alloc_register(f"widx_reg{i}")
                     for i in range(2 * S)]

            def load_widx_span(h):
                nc.tensor.reg_load(wregs[16 * h: 16 * h + 16],
                                   widx_sb[0:1, 16 * h: 16 * h + 16])

            def rv_of(s):
                a = RuntimeValue(val=wregs[2 * s], min_val=0, max_val=WMAX)
                b = RuntimeValue(val=wregs[2 * s + 1], min_val=0,
                                 max_val=WMAX)
                return a, b

            def load_x(s):
                x_sb = xpool.tile([128, XCOLS], bf16, tag="x",
                                  name=f"x_sb_{s}")
                nc.sync.dma_start(out=x_sb[:, :], in_=x_h[s, :, :])
                return x_sb

            def scale_x(s, x_sb):
                # xs[:, 0:1024] = g0 * x ; xs[:, 1024:2048] = g1 * x
                xs = spool.tile([128, 2 * XCOLS], bf16, tag="xs",
                                name=f"xs_{s}")
                nc.vector.tensor_scalar_mul(
                    xs[:, 0: XCOLS], x_sb[:, :], g_sb[:, 2 * s: 2 * s + 1])
                nc.vector.tensor_scalar_mul(
                    xs[:, XCOLS: 2 * XCOLS], x_sb[:, :],
                    g_sb[:, 2 * s + 1: 2 * s + 2])
                return xs

            def mm_tail(ps, xt, rv, p0, stop):
                # K=5 tail at partition offset p0, row-tiled
                nc.tensor.matmul(
                    ps[:, :],
                    xt[p0: p0 + K_LAST, 7 * 128: XCOLS],
                    w_sb[7][p0: p0 + K_LAST, bass.ds(rv, D_MODEL)],
                    start=False, stop=stop,
                    tile_position=(p0, 0),
                )

            # ---------- phase 1: samples 0..7, scaled one-bank k-outer ----
            p1_x = [load_x(0)]
            load_w(0)
            load_w(1)
            p1_x += [load_x(s) for s in range(1, 4)]
            load_w(2)
            p1_x += [load_x(s) for s in range(4, 6)]
            load_w(3)
            p1_x += [load_x(s) for s in range(6, 8)]
            for k in range(4, 7):
                load_w(k)
            load_w_tail()

            load_widx_span(0)
            # HAM warmup: ~3.4us of dummy fp32 matmuls on the gates tile
            # while the real DMAs land (PE would otherwise start at 1.2 GHz)
            ps_warm = pspool.tile([128, D_MODEL], f32, tag="ps",
                                  name="ps_warm")
            for _ in range(8):
                nc.tensor.matmul(ps_warm[:, 0:128], g_sb[:, 0:128],
                                 g_sb[:, 0:128], start=True, stop=True)

            p1_xs = [scale_x(s, p1_x[s]) for s in range(P1)]
            p1_ps = [pspool.tile([128, D_MODEL], f32, tag="ps",
                                 name=f"ps_{s}") for s in range(P1)]
            p1_rv = [rv_of(s) for s in range(P1)]
            for k in range(7):
                for s in range(P1):       # expert-A sweep (banks cycle 0..7)
                    nc.tensor.matmul(
                        p1_ps[s][:, :],
                        p1_xs[s][:, k * 128: (k + 1) * 128],
                        w_sb[k][:, bass.ds(p1_rv[s][0], D_MODEL)],
                        start=(k == 0), stop=False,
                    )
                for s in range(P1):       # expert-B sweep
                    nc.tensor.matmul(
                        p1_ps[s][:, :],
                        p1_xs[s][:, XCOLS + k * 128: XCOLS + (k + 1) * 128],
                        w_sb[k][:, bass.ds(p1_rv[s][1], D_MODEL)],
                        start=False, stop=False,
                    )
                load_widx_span(k + 1)     # hide reg loads in DMA-bound phase
            for grp in range(2):
                for j in range(4):
                    s = grp * 4 + j
                    mm_tail(p1_ps[s], p1_xs[s][:, 0: XCOLS],
                            p1_rv[s][0], 32 * j, stop=False)
                for j in range(4):
                    s = grp * 4 + j
                    mm_tail(p1_ps[s], p1_xs[s][:, XCOLS: 2 * XCOLS],
                            p1_rv[s][1], 32 * j, stop=True)
            for s in range(P1):
                o_sb = opool.tile([128, D_MODEL], bf16, tag="o",
                                  name=f"o_{s}")
                # alternate engines so PSUM banks free 2-at-a-time for
                # the phase-2 pipeline start
                if s % 2 == 0:
                    nc.scalar.copy(o_sb[:, :], p1_ps[s][:, :])
                else:
                    nc.vector.tensor_copy(o_sb[:, :], p1_ps[s][:, :])
                nc.gpsimd.dma_start(out=y_h[s, :, :], in_=o_sb[:, :])

            # ---------- phase 2: sample pairs, shared-stationary 2-bank ----
            for pair in range(P1 // 2, S // 2):
                s0 = 2 * pair
                xp, psA, psB, rvp = [], [], [], []
                for j in range(2):
                    s = s0 + j
                    xp.append(load_x(s))
                    psA.append(pspool.tile([128, D_MODEL], f32, tag="ps",
                                           name=f"psA_{s}"))
                    psB.append(pspool.tile([128, D_MODEL], f32, tag="ps",
                                           name=f"psB_{s}"))
                    rvp.append(rv_of(s))
                for j in range(2):
                    rvA, rvB = rvp[j]
                    for k in range(7):
                        # shared stationary x chunk, banks alternate A/B
                        nc.tensor.matmul(
                            psA[j][:, :],
                            xp[j][:, k * 128: (k + 1) * 128],
                            w_sb[k][:, bass.ds(rvA, D_MODEL)],
                            start=(k == 0), stop=False,
                        )
                        nc.tensor.matmul(
                            psB[j][:, :],
                            xp[j][:, k * 128: (k + 1) * 128],
                            w_sb[k][:, bass.ds(rvB, D_MODEL)],
                            start=(k == 0), stop=False,
                        )
                # tails: row groups 0 / 32; A-sweep then B-sweep so the two
                # row groups overlap (matmul starts are pc-monotone)
                for j in range(2):
                    mm_tail(psA[j], xp[j], rvp[j][0], 32 * j, stop=True)
                for j in range(2):
                    mm_tail(psB[j], xp[j], rvp[j][1], 32 * j, stop=True)
                for j in range(2):
                    s = s0 + j
                    t1 = tpool.tile([128, D_MODEL], f32, tag="t",
                                    name=f"t1_{s}")
                    nc.scalar.mul(t1[:, :], psB[j][:, :],
                                  g_sb[:, 2 * s + 1: 2 * s + 2])
                    o_sb = opool.tile([128, D_MODEL], bf16, tag="o",
                                      name=f"o_{s}")
                    # o = gA * psA + t1  (one fused DVE pass)
                    nc.vector.scalar_tensor_tensor(
                        o_sb[:, :], psA[j][:, :],
                        g_sb[:, 2 * s: 2 * s + 1], t1[:, :],
                        mybir.AluOpType.mult, mybir.AluOpType.add,
                    )
                    nc.gpsimd.dma_start(out=y_h[s, :, :], in_=o_sb[:, :])

    nc.finalize()
    return nc


def _gates_np(logits, moe_masks):
    """Mirror reference _gates in numpy (fp32)."""
    lg = logits.astype(np.float32)
    m = lg.max(axis=1, keepdims=True)
    e = np.exp(lg - m)
    g = e / e.sum(axis=1, keepdims=True)
    g = g * (moe_masks == 1).astype(np.float32)
    # top-2, ties -> lower index first (matches jax.lax.top_k)
    top_idx = np.argsort(-g, axis=1, kind="stable")[:, :TOP_K]
    rows = np.arange(g.shape[0])[:, None]
    gsel = g[rows, top_idx]                                  # [B, 2]
    gsel = gsel / (gsel.sum(axis=1, keepdims=True) + EPS)
    return gsel.astype(np.float32), top_idx.astype(np.int32)


def _prep_inputs(cycle_curve_data, logits, moe_masks, W, b):
    gsel, top_idx = _gates_np(logits, moe_masks)

    xf = cycle_curve_data.reshape(B, L, FEAT).astype(np.float32, copy=False)
    # x_host[s, p, k*128 + l] = x[s, l, k*128 + p] for chunks 0..6;
    # tail rows 896..899 + bias row at the per-sample partition offset
    x_host = np.zeros((B, 128, XCOLS), np.float32)
    x_host[:, :, : 7 * 128] = np.ascontiguousarray(
        xf[:, :, : 7 * 128].reshape(B, L, 7, 128).transpose(0, 3, 2, 1)
    ).reshape(B, 128, 7 * 128)
    tail = np.empty((B, K_LAST, L), np.float32)
    tail[:, :4, :] = xf[:, :, 7 * 128: FEAT].transpose(0, 2, 1)
    tail[:, 4, :] = 1.0                                      # bias row
    poff = np.array([_tail_poff(s % S) for s in range(B)])
    for p0 in (0, 32, 64, 96):
        m = poff == p0
        x_host[m, p0: p0 + K_LAST, 7 * 128:] = tail[m]
    x_host = x_host.astype(ml_dtypes.bfloat16)

    w_aug = np.concatenate(
        [W.astype(np.float32), b.astype(np.float32)[:, None, :]], axis=1
    )                                                        # [E, 901, 512]
    w_host = np.zeros((N_KCH, 128, NUM_EXPERTS, D_MODEL), np.float32)
    for k in range(7):
        w_host[k] = w_aug[:, k * 128: (k + 1) * 128, :].transpose(1, 0, 2)
    for j in range(4):
        w_host[7, 32 * j: 32 * j + K_LAST] = \
            w_aug[:, 7 * 128:, :].transpose(1, 0, 2)
    w_host = w_host.astype(ml_dtypes.bfloat16)

    in_maps = []
    for c in range(N_CORES):
        sl = slice(c * S, (c + 1) * S)
        g_rep = np.broadcast_to(
            gsel[sl].reshape(1, 2 * S), (128, 2 * S)
        ).copy()
        widx = (top_idx[sl].reshape(1, 2 * S) * D_MODEL).astype(np.int32)
        in_maps.append({
            "x": np.ascontiguousarray(x_host[sl]),
            "w": w_host,
            "g": g_rep,
            "widx": widx,
        })
    return in_maps


def kernel(cycle_curve_data, logits, moe_masks, W, b):
    if "nc" not in _CACHE:
        _CACHE["nc"] = _build_nc()
    nc = _CACHE["nc"]

    in_maps = _prep_inputs(cycle_curve_data, logits, moe_masks, W, b)

    trace = bool(int(os.environ.get("KERNEL_PROFILE", "0")))
    res = run_bass_kernel_spmd(
        nc, in_maps, core_ids=list(range(N_CORES)), trace=trace
    )
    _CACHE["last_results"] = res

    out = np.empty((B, L, D_MODEL), ml_dtypes.bfloat16)
    for c in range(N_CORES):
        out[c * S: (c + 1) * S] = res.results[c]["y"]
    return out


# revision 12
# speedup vs baseline: 1.4055x; 1.0315x over previous
"""Trainium2 Bass kernel for BatteryMoEFlattenIntraCycleMoELayer.

Computation (reference):
    gates = renorm(top2(softmax(logits) * mask))          # [B, E]
    x = cycle_curve_data.reshape(B, L, 900)
    out[b] = sum_e gates[b,e] * (x[b] @ W[e] + b[e])      # -> bf16 [B, L, 512]

Strategy (v4):
  - Host: gates + top-2 routing (tiny). x transposed feat-major into
    [B, 128, 1024] bf16: cols 0..895 = chunks 0..6 (partition =
    feature-within-chunk), cols 896..1023 = the 5-row tail (4 data rows +
    constant-1.0 bias row; bias folds into the matmul via W_aug = [W; b])
    at a per-sample partition offset so neighboring samples' tails run as
    concurrent row-tiled matmuls.
  - Shard B across 8 cores (64 samples each). One SPMD program: routing is
    data (per-sample W-slot offsets in PE registers, dynamic APs on the
    moving operand). All 128 offsets are register-resident: 8 TENSOR_LOADs
    tucked into the DMA-bound phase 1, so steady state never stalls.
  - Dummy fp32 matmuls on the gates tile warm the PE HAM clock gate during
    the initial DMA wait (otherwise the first ~3.4us of real matmuls run at
    1.2 GHz instead of 2.4).
  - Phase 1 (samples 0..7): DVE pre-scales x by the gates (xA=g0*x, xB=g1*x)
    so both experts accumulate into ONE PSUM bank per sample; k-outer over
    8 samples paces the PE against the streaming weight DMAs. One
    ACT-or-DVE copy per sample.
  - Phase 2 (steady): pairs of samples, UNSCALED x stationary shared by the
    two expert matmuls (one LDWEIGHTS per 2 matmuls) alternating two PSUM
    banks (avoids the same-bank accumulation stall); combine = ACT mul
    (t=g1*psB) + DVE scalar_tensor_tensor (o = g0*psA + t, bf16). Tails of
    the 2 samples run row-tiled at partition offsets 0/32.
  - DMA discipline: each dma_start costs ~650ns of Sync-queue issue time,
    so transfers are few and big, issued in consumption order; y outputs
    go out via the otherwise-idle GpSimd (SWDGE) queue.
"""

import os
import sys

for _p in ("/opt/trn_rl_repo", "/root/.axon_site/_ro/trn_rl_repo"):
    if os.path.isdir(_p) and _p not in sys.path:
        sys.path.insert(0, _p)

import numpy as np
import ml_dtypes

import concourse.bass as bass
import concourse.mybir as mybir
import concourse.tile as tile
from concourse import bacc
from concourse.bass_utils import run_bass_kernel_spmd
from concourse.bass_values import RuntimeValue

B, L, CURVE_LEN = 512, 128, 300
FEAT = 3 * CURVE_LEN          # 900
FEAT_AUG = FEAT + 1           # 901 (bias row)
D_MODEL = 512
NUM_EXPERTS = 8
TOP_K = 2
EPS = 1e-9
N_CORES = 8
S = B // N_CORES              # 64 samples per core
N_KCH = 8                     # K chunks: 7 x 128 + 1 x 5 (tail incl bias)
K_LAST = FEAT_AUG - 7 * 128   # 5
XCOLS = 1024                  # 7*128 main cols + 128 tail cols
WMAX = (NUM_EXPERTS - 1) * D_MODEL
P1 = 8                        # phase-1 sample count

_CACHE = {}


def _tail_poff(s):
    """Partition offset of sample s's tail rows (row-group for tiling)."""
    return 32 * (s % 4) if s < P1 else 32 * (s % 2)


def _build_nc():
    """Build the SPMD Bass program (routing-independent)."""
    nc = bacc.Bacc(trn_type="TRN2")
    f32 = mybir.dt.float32
    bf16 = mybir.dt.bfloat16
    i32 = mybir.dt.int32

    x_h = nc.declare_dram_parameter("x", [S, 128, XCOLS], bf16, isOutput=False)
    # w per k-chunk: [k, part(<=128), expert, 512]; k=7 holds the 5 tail
    # rows replicated at partition offsets 0/32/64/96
    w_h = nc.declare_dram_parameter("w", [N_KCH, 128, NUM_EXPERTS, D_MODEL],
                                    bf16, isOutput=False)
    g_h = nc.declare_dram_parameter("g", [128, 2 * S], f32, isOutput=False)
    widx_h = nc.declare_dram_parameter("widx", [1, 2 * S], i32, isOutput=False)
    y_h = nc.declare_dram_parameter("y", [S, L, D_MODEL], bf16, isOutput=True)

    with tile.TileContext(nc) as tc:
        with (
            tc.tile_pool(name="cpool", bufs=1) as cpool,
            tc.tile_pool(name="xpool", bufs=12) as xpool,
            tc.tile_pool(name="spool", bufs=8) as spool,
            tc.tile_pool(name="tpool", bufs=4) as tpool,
            tc.tile_pool(name="opool", bufs=4) as opool,
            tc.tile_pool(name="pspool", bufs=8, space="PSUM") as pspool,
        ):
            # --- constants; issue order == consumption order ---
            widx_sb = cpool.tile([1, 2 * S], i32)
            g_sb = cpool.tile([128, 2 * S], f32)

            w_sb = [cpool.tile([128, NUM_EXPERTS * D_MODEL], bf16,
                               name=f"w_sb_{k}") for k in range(N_KCH)]

            def load_w(k):
                nc.sync.dma_start(
                    out=w_sb[k][:, :],
                    in_=w_h[k, :, :, :].rearrange("p e d -> p (e d)"),
                )

            def load_w_tail():
                # only the 4 replicated 5-row strips of k=7 carry data
                for j in range(4):
                    p0 = 32 * j
                    nc.sync.dma_start(
                        out=w_sb[7][p0: p0 + K_LAST, :],
                        in_=w_h[7, p0: p0 + K_LAST, :, :].rearrange(
                            "p e d -> p (e d)"),
                    )

            # all 128 per-sample W-slot offsets stay register-resident
            wregs = [nc.tensor.alloc_register(f"widx_reg{i}")
                     for i in range(2 * S)]

            def load_widx_span(h):
                nc.tensor.reg_load(wregs[16 * h: 16 * h + 16],
                                   widx_sb[0:1, 16 * h: 16 * h + 16])

            def rv_of(s):
                a = RuntimeValue(val=wregs[2 * s], min_val=0, max_val=WMAX)
                b = RuntimeValue(val=wregs[2 * s + 1], min_val=0,
                                 max_val=WMAX)
                return a, b

            def load_x(s):
                x_sb = xpool.tile([128, XCOLS], bf16, tag="x",
                                  name=f"x_sb_{s}")
                nc.sync.dma_start(out=x_sb[:, :], in_=x_h[s, :, :])
                return x_sb

            def scale_x(s, x_sb):
                # xs[:, 0:1024] = g0 * x ; xs[:, 1024:2048] = g1 * x
                xs = spool.tile([128, 2 * XCOLS], bf16, tag="xs",
                                name=f"xs_{s}")
                nc.vector.tensor_scalar_mul(
                    xs[:, 0: XCOLS], x_sb[:, :], g_sb[:, 2 * s: 2 * s + 1])
                nc.vector.tensor_scalar_mul(
                    xs[:, XCOLS: 2 * XCOLS], x_sb[:, :],
                    g_sb[:, 2 * s + 1: 2 * s + 2])
                return xs

            def mm_tail(ps, xt, rv, p0, stop):
                # K=5 tail at partition offset p0, row-tiled
                nc.tensor.matmul(
                    ps[:, :],
                    xt[p0: p0 + K_LAST, 7 * 128: XCOLS],
                    w_sb[7][p0: p0 + K_LAST, bass.ds(rv, D_MODEL)],
                    start=False, stop=stop,
                    tile_position=(p0, 0),
                )

            # ---------- phase 1: samples 0..7, scaled one-bank k-outer ----
            load_w(0)
            nc.sync.dma_start(out=widx_sb[:, :], in_=widx_h[:, :])
            nc.sync.dma_start(out=g_sb[:, :], in_=g_h[:, :])
            p1_x = [load_x(s) for s in range(4)]
            load_w(1)
            p1_x += [load_x(s) for s in range(4, 8)]
            for k in range(2, 7):
                load_w(k)
            load_w_tail()

            load_widx_span(0)
            # HAM warmup: ~3.4us of dummy fp32 matmuls on the gates tile
            # while the real DMAs land (PE would otherwise start at 1.2 GHz)
            ps_warm = pspool.tile([128, D_MODEL], f32, tag="ps",
                                  name="ps_warm")
            for _ in range(8):
                nc.tensor.matmul(ps_warm[:, 0:128], g_sb[:, 0:128],
                                 g_sb[:, 0:128], start=True, stop=True)

            p1_xs = [scale_x(s, p1_x[s]) for s in range(P1)]
            p1_ps = [pspool.tile([128, D_MODEL], f32, tag="ps",
                                 name=f"ps_{s}") for s in range(P1)]
            p1_rv = [rv_of(s) for s in range(P1)]
            for k in range(7):
                for s in range(P1):       # expert-A sweep (banks cycle 0..7)
                    nc.tensor.matmul(
                        p1_ps[s][:, :],
                        p1_xs[s][:, k * 128: (k + 1) * 128],
                        w_sb[k][:, bass.ds(p1_rv[s][0], D_MODEL)],
                        start=(k == 0), stop=False,
                    )
                for s in range(P1):       # expert-B sweep
                    nc.tensor.matmul(
                        p1_ps[s][:, :],
                        p1_xs[s][:, XCOLS + k * 128: XCOLS + (k + 1) * 128],
                        w_sb[k][:, bass.ds(p1_rv[s][1], D_MODEL)],
                        start=False, stop=False,
                    )
                load_widx_span(k + 1)     # hide reg loads in DMA-bound phase
            for grp in range(2):
                for j in range(4):
                    s = grp * 4 + j
                    mm_tail(p1_ps[s], p1_xs[s][:, 0: XCOLS],
                            p1_rv[s][0], 32 * j, stop=False)
                for j in range(4):
                    s = grp * 4 + j
                    mm_tail(p1_ps[s], p1_xs[s][:, XCOLS: 2 * XCOLS],
                            p1_rv[s][1], 32 * j, stop=True)
            for s in range(P1):
                o_sb = opool.tile([128, D_MODEL], bf16, tag="o",
                                  name=f"o_{s}")
                # alternate engines so PSUM banks free 2-at-a-time for
                # the phase-2 pipeline start
                if s % 2 == 0:
                    nc.scalar.copy(o_sb[:, :], p1_ps[s][:, :])
                else:
                    nc.vector.tensor_copy(o_sb[:, :], p1_ps[s][:, :])
                nc.scalar.dma_start(out=y_h[s, :, :], in_=o_sb[:, :])

            # ---------- phase 2: sample pairs, shared-stationary 2-bank ----
            for pair in range(P1 // 2, S // 2):
                s0 = 2 * pair
                xp, psA, psB, rvp = [], [], [], []
                for j in range(2):
                    s = s0 + j
                    xp.append(load_x(s))
                    psA.append(pspool.tile([128, D_MODEL], f32, tag="ps",
                                           name=f"psA_{s}"))
                    psB.append(pspool.tile([128, D_MODEL], f32, tag="ps",
                                           name=f"psB_{s}"))
                    rvp.append(rv_of(s))
                for j in range(2):
                    rvA, rvB = rvp[j]
                    for k in range(7):
                        # shared stationary x chunk, banks alternate A/B
                        nc.tensor.matmul(
                            psA[j][:, :],
                            xp[j][:, k * 128: (k + 1) * 128],
                            w_sb[k][:, bass.ds(rvA, D_MODEL)],
                            start=(k == 0), stop=False,
                        )
                        nc.tensor.matmul(
                            psB[j][:, :],
                            xp[j][:, k * 128: (k + 1) * 128],
                            w_sb[k][:, bass.ds(rvB, D_MODEL)],
                            start=(k == 0), stop=False,
                        )
                # tails: row groups 0 / 32; A-sweep then B-sweep so the two
                # row groups overlap (matmul starts are pc-monotone)
                for j in range(2):
                    mm_tail(psA[j], xp[j], rvp[j][0], 32 * j, stop=True)
                for j in range(2):
                    mm_tail(psB[j], xp[j], rvp[j][1], 32 * j, stop=True)
                for j in range(2):
                    s = s0 + j
                    t1 = tpool.tile([128, D_MODEL], f32, tag="t",
                                    name=f"t1_{s}")
                    nc.scalar.mul(t1[:, :], psB[j][:, :],
                                  g_sb[:, 2 * s + 1: 2 * s + 2])
                    o_sb = opool.tile([128, D_MODEL], bf16, tag="o",
                                      name=f"o_{s}")
                    # o = gA * psA + t1  (one fused DVE pass)
                    nc.vector.scalar_tensor_tensor(
                        o_sb[:, :], psA[j][:, :],
                        g_sb[:, 2 * s: 2 * s + 1], t1[:, :],
                        mybir.AluOpType.mult, mybir.AluOpType.add,
                    )
                    nc.scalar.dma_start(out=y_h[s, :, :], in_=o_sb[:, :])

    nc.finalize()
    return nc


def _gates_np(logits, moe_masks):
    """Mirror reference _gates in numpy (fp32)."""
    lg = logits.astype(np.float32)
    m = lg.max(axis=1, keepdims=True)
    e = np.exp(lg - m)
    g = e / e.sum(axis=1, keepdims=True)
    g = g * (moe_masks == 1).astype(np.float32)
    # top-2, ties -> lower index first (matches jax.lax.top_k)
    top_idx = np.argsort(-g, axis=1, kind="stable")[:, :TOP_K]
    rows = np.arange(g.shape[0])[:, None]
    gsel = g[rows, top_idx]                                  # [B, 2]
    gsel = gsel / (gsel.sum(axis=1, keepdims=True) + EPS)
    return gsel.astype(np.float32), top_idx.astype(np.int32)


def _prep_inputs(cycle_curve_data, logits, moe_masks, W, b):
    gsel, top_idx = _gates_np(logits, moe_masks)

    xf = cycle_curve_data.reshape(B, L, FEAT).astype(np.float32, copy=False)
    # x_host[s, p, k*128 + l] = x[s, l, k*128 + p] for chunks 0..6;
    # tail rows 896..899 + bias row at the per-sample partition offset
    x_host = np.zeros((B, 128, XCOLS), np.float32)
    x_host[:, :, : 7 * 128] = np.ascontiguousarray(
        xf[:, :, : 7 * 128].reshape(B, L, 7, 128).transpose(0, 3, 2, 1)
    ).reshape(B, 128, 7 * 128)
    tail = np.empty((B, K_LAST, L), np.float32)
    tail[:, :4, :] = xf[:, :, 7 * 128: FEAT].transpose(0, 2, 1)
    tail[:, 4, :] = 1.0                                      # bias row
    poff = np.array([_tail_poff(s % S) for s in range(B)])
    for p0 in (0, 32, 64, 96):
        m = poff == p0
        x_host[m, p0: p0 + K_LAST, 7 * 128:] = tail[m]
    x_host = x_host.astype(ml_dtypes.bfloat16)

    w_aug = np.concatenate(
        [W.astype(np.float32), b.astype(np.float32)[:, None, :]], axis=1
    )                                                        # [E, 901, 512]
    w_host = np.zeros((N_KCH, 128, NUM_EXPERTS, D_MODEL), np.float32)
    for k in range(7):
        w_host[k] = w_aug[:, k * 128: (k + 1) * 128, :].transpose(1, 0, 2)
    for j in range(4):
        w_host[7, 32 * j: 32 * j + K_LAST] = \
            w_aug[:, 7 * 128:, :].transpose(1, 0, 2)
    w_host = w_host.astype(ml_dtypes.bfloat16)

    in_maps = []
    for c in range(N_CORES):
        sl = slice(c * S, (c + 1) * S)
        g_rep = np.broadcast_to(
            gsel[sl].reshape(1, 2 * S), (128, 2 * S)
        ).copy()
        widx = (top_idx[sl].reshape(1, 2 * S) * D_MODEL).astype(np.int32)
        in_maps.append({
            "x": np.ascontiguousarray(x_host[sl]),
            "w": w_host,
            "g": g_rep,
            "widx": widx,
        })
    return in_maps


def kernel(cycle_curve_data, logits, moe_masks, W, b):
    if "nc" not in _CACHE:
        _CACHE["nc"] = _build_nc()
    nc = _CACHE["nc"]

    in_maps = _prep_inputs(cycle_curve_data, logits, moe_masks, W, b)

    trace = bool(int(os.environ.get("KERNEL_PROFILE", "0")))
    res = run_bass_kernel_spmd(
        nc, in_maps, core_ids=list(range(N_CORES)), trace=trace
    )
    _CACHE["last_results"] = res

    out = np.empty((B, L, D_MODEL), ml_dtypes.bfloat16)
    for c in range(N_CORES):
        out[c * S: (c + 1) * S] = res.results[c]["y"]
    return out
